# revision 18
# baseline (speedup 1.0000x reference)
"""Causal self-attention (B=2, T=2048, C=1024, H=16) on 8 trn2 NeuronCores.

Sharding (Megatron-style over heads):
  - tensor-parallel over heads: core p owns heads {2p, 2p+1}.  Each core
    computes Q^T/K^T/V^T for its 2 heads from the full x, then causal
    attention (streaming softmax without max-subtraction; the denominator
    comes from a ones-column appended to V).
  - per 512-token q-chunk, an AllToAll redistributes that chunk's
    normalized attention outputs so core p holds all 1024 channels for
    tokens [512*g + 64*p, 512*g + 64*p + 64); the output projection runs
    per 128-token part (2 chunks) with the full W_proj.
  - projection output: core p writes a disjoint [512, 1024] block; rows
    b*256 + part*128 + gg*64 + t = batch b, token 512*(2*part+gg) + 64*p + t.

Scheduling: one software-pipelined instruction stream.  Attention per
q-chunk emits S(ki) -> [filler unit] -> exp(ki) -> mask -> PV(ki); the
filler units are the next batch's qkv chunks and earlier parts' output
projections, so the (in-order) PE never waits on the exp pipeline.  Exp
is sliced to the causal range [lo:512].  The softmax normalization is
fused into the PSUM->SBUF copy (o_t * recip broadcast -> anorm bf16).
DMA queues: bulk streams + recip bounce on sync, afull/y on scalar,
a2a staging + collectives on gpsimd.
"""

import numpy as np

B, T, C, H, D = 2, 2048, 1024, 16, 64
NCORES = 8
HL = H // NCORES        # heads per core = 2
TOK = B * T             # 4096 global tokens
TSL = TOK // NCORES     # 512 output tokens per core (256 per batch)
SL = 256                # per-batch token slice per core
P = 128
CT = C // P             # 8 contraction tiles
SCALE = D ** -0.5

_CACHE = {}


def _build_nc():
    import concourse.bass as bass
    import concourse.mybir as mybir
    from concourse import bacc
    from concourse.tile import TileContext

    f32 = mybir.dt.float32
    bf16 = mybir.dt.bfloat16
    AF = mybir.ActivationFunctionType
    ALU = mybir.AluOpType

    nc = bacc.Bacc(
        "TRN2", target_bir_lowering=False, debug=False, num_devices=NCORES
    )

    xT = nc.dram_tensor("xT", [C, TOK], bf16, kind="ExternalInput")
    wqkvT = nc.dram_tensor("wqkvT", [C, 3 * P], bf16, kind="ExternalInput")
    bqkv = nc.dram_tensor("bqkv", [3 * P], f32, kind="ExternalInput")
    wpT = nc.dram_tensor("wpT", [C, C], bf16, kind="ExternalInput")
    bp = nc.dram_tensor("bp", [C], bf16, kind="ExternalInput")
    tri = nc.dram_tensor("tri", [P, P], bf16, kind="ExternalInput")
    onesd = nc.dram_tensor("ones", [P, P], bf16, kind="ExternalInput")
    ident = nc.dram_tensor("ident", [P, P], bf16, kind="ExternalInput")
    y = nc.dram_tensor("y", [TSL, C], f32, kind="ExternalOutput")

    with TileContext(nc, num_cores=NCORES) as tc:
        from contextlib import ExitStack

        with ExitStack() as ctx:
            const = ctx.enter_context(tc.tile_pool(name="const", bufs=1))
            persist = ctx.enter_context(tc.tile_pool(name="persist", bufs=1))
            dram = ctx.enter_context(tc.tile_pool(name="dram", bufs=1, space="DRAM"))

            # ---- constants; small ones first so nothing queues behind bulk
            tri_sb = const.tile([P, P], bf16)
            id_sb = const.tile([P, P], bf16)
            bq_sb = const.tile([P, 3], f32)
            bp_sb = const.tile([1, C], bf16)
            ones_sb = const.tile([1, P], bf16)
            ones2_sb = const.tile([P, 2], bf16)
            w_sb = const.tile([P, CT, 3 * P], bf16)     # wqkvT tiles
            wp_sb = const.tile([P, CT, C], bf16)        # W_proj^T (loaded late)
            nc.gpsimd.dma_start(tri_sb[:], tri[:])
            nc.gpsimd.dma_start(id_sb[:], ident[:])
            nc.gpsimd.dma_start(bq_sb[:], bqkv.rearrange("(et p) -> p et", p=P))
            nc.gpsimd.dma_start(bp_sb[:], bp.rearrange("(o c) -> o c", o=1))
            nc.gpsimd.dma_start(ones_sb[:], onesd[0:1, :])
            nc.gpsimd.dma_start(ones2_sb[:], onesd[:, 0:2])
            nc.sync.dma_start(w_sb[:], wqkvT.rearrange("(ct p) e -> p ct e", p=P))

            # ---- persistent activations (per batch for fine-grained deps)
            qTb = [persist.tile([P, T], bf16, name=f"qT{b}") for b in range(B)]
            kTb = [persist.tile([P, T], bf16, name=f"kT{b}") for b in range(B)]
            vTb = [persist.tile([P, T], bf16, name=f"vT{b}") for b in range(B)]
            vaugb = [persist.tile([P, T // P, 2 * 65], bf16, name=f"vaug{b}")
                     for b in range(B)]
            anorm = [persist.tile([64, TOK], bf16, name=f"anorm{h}")
                     for h in range(HL)]
            rdram = dram.tile([B * HL, T], f32)          # reciprocals (bounce)

            pools = [
                tc.tile_pool(name="sps", bufs=2, space="PSUM"),
                tc.tile_pool(name="ops", bufs=2, space="PSUM"),
                tc.tile_pool(name="mm", bufs=2, space="PSUM"),
                tc.tile_pool(name="pT", bufs=2),
                tc.tile_pool(name="rp", bufs=2),
                tc.tile_pool(name="rb", bufs=2),
                tc.tile_pool(name="xslab", bufs=3),
                tc.tile_pool(name="afull", bufs=4),
                tc.tile_pool(name="ysb", bufs=2),
            ]
            (sps, ops, mm, ppool, rppool, rbpool, xpool, apool,
             ypool) = (ctx.enter_context(p) for p in pools)

            def qkv_units(b, c):
                """Filler units computing qkv^T for one 512-token chunk."""
                t0 = b * T + c * 512
                state = {}
                units = []

                def load():
                    xsl = xpool.tile([P, CT, 512], bf16, tag="x",
                                     name=f"x{b}{c}")
                    state["x"] = xsl
                    xv = xT[:, t0:t0 + 512].rearrange(
                        "(ct p) t -> p ct t", p=P)
                    nc.sync.dma_start(xsl[:, 0:4, :], xv[:, 0:4, :])
                    nc.sync.dma_start(xsl[:, 4:8, :], xv[:, 4:8, :])
                units.append(load)

                def mk_et(et, dstl):
                    def u():
                        ps = mm.tile([P, 512], f32, tag="mm")
                        for ct in range(CT):
                            nc.tensor.matmul(
                                ps[:],
                                lhsT=w_sb[:, ct, et * P:(et + 1) * P],
                                rhs=state["x"][:, ct, :],
                                start=(ct == 0),
                                stop=(ct == CT - 1),
                            )
                        nc.vector.tensor_scalar_add(
                            dstl[b][:, c * 512:(c + 1) * 512],
                            ps[:],
                            bq_sb[:, et:et + 1],
                        )
                    return u
                for et, dstl in enumerate((qTb, kTb, vTb)):
                    units.append(mk_et(et, dstl))

                def mk_tr(half):
                    def u():
                        for kt in range(c * 4 + half * 2, c * 4 + half * 2 + 2):
                            tp = mm.tile([P, P], bf16, tag="mm")
                            nc.tensor.transpose(
                                tp[:], vTb[b][:, kt * P:(kt + 1) * P], id_sb[:]
                            )
                            nc.vector.tensor_copy(
                                vaugb[b][:, kt, 0:2 * 65]
                                .rearrange("p (h e) -> p h e", h=2)[:, :, 0:64],
                                tp.rearrange("p (h e) -> p h e", h=2),
                            )
                            nc.vector.tensor_copy(
                                vaugb[b][:, kt, 64:2 * 65:65], ones2_sb[:]
                            )
                    return u
                for half in (0, 1):
                    units.append(mk_tr(half))
                return units

            def att(b, qc, fillers):
                """Causal attention for one 512-token q-chunk, weaving
                filler units between the score and PV matmul pairs."""
                q0 = qc * 512
                nk = 4 * qc + 4
                o_t = [ops.tile([65, 512], f32, tag="o", name=f"ot{h}")
                       for h in range(HL)]
                fi = 0
                for ki in range(nk):
                    sp = sps.tile([P, HL, 512], f32, tag="s")
                    for h in range(HL):
                        hp = slice(64 * h, 64 * h + 64)
                        nc.tensor.matmul(
                            sp[:, h, :],
                            lhsT=kTb[b][hp, ki * P:(ki + 1) * P],
                            rhs=qTb[b][hp, q0:q0 + 512],
                            start=True,
                            stop=True,
                        )
                    for _ in range(2 if ki == 0 else 1):
                        if fi < len(fillers):
                            fillers[fi]()
                            fi += 1
                    off = ki * P - q0
                    lo = max(0, off)
                    pt = ppool.tile([P, HL, 512], bf16, tag="p")
                    nc.scalar.activation(
                        pt[:, :, lo:512], sp[:, :, lo:512], AF.Exp, scale=SCALE,
                    )
                    for h in range(HL):
                        if off >= 0:
                            nc.vector.tensor_tensor(
                                pt[:, h, off:off + P],
                                pt[:, h, off:off + P],
                                tri_sb[:],
                                ALU.mult,
                            )
                        nc.tensor.matmul(
                            o_t[h][:, lo:512],
                            lhsT=vaugb[b][:, ki, h * 65:h * 65 + 65],
                            rhs=pt[:, h, lo:512],
                            start=(ki == 0),
                            stop=(ki == nk - 1),
                        )
                while fi < len(fillers):
                    fillers[fi]()
                    fi += 1
                return o_t

            def epi(b, qc, o_t):
                """Per-chunk softmax normalization: recip of the denominator
                rows, batched over both heads (DRAM bounce for the partition
                reshape+broadcast), then a fused multiply during the
                PSUM->SBUF copy."""
                q0 = qc * 512
                c0 = b * T + q0
                bh0 = b * HL
                den2 = rppool.tile([33, 512], f32, tag="den")
                for h in range(HL):
                    nc.vector.tensor_copy(
                        den2[32 * h:32 * h + 1, :], o_t[h][64:65, :])
                dpk = rppool.tile([16, 64], f32, tag="dpk")
                rpk = rppool.tile([16, 64], f32, tag="rpk")
                rsc = rppool.tile([16, 64], f32, tag="rsc")
                for h in range(HL):
                    nc.sync.dma_start(
                        dpk[8 * h:8 * h + 8, :], den2[32 * h:32 * h + 1, :])
                nc.vector.reciprocal_approx_accurate(rpk[:], dpk[:], rsc[:])
                for h in range(HL):
                    nc.sync.dma_start(
                        rdram[bh0 + h:bh0 + h + 1, q0:q0 + 512]
                        .rearrange("o (rr f) -> (o rr) f", f=64),
                        rpk[8 * h:8 * h + 8, :],
                    )
                for h in range(HL):
                    rb = rbpool.tile([64, 512], f32, tag="rb")
                    nc.sync.dma_start(
                        rb[:],
                        rdram[bh0 + h:bh0 + h + 1, q0:q0 + 512]
                        .to_broadcast((64, 512)),
                    )
                    nc.vector.tensor_tensor(
                        anorm[h][:, c0:c0 + 512],
                        o_t[h][0:64, :],
                        rb[:],
                        ALU.mult,
                    )

            afull = {}

            def a2a(b, g0, ng):
                """AllToAll for `ng` 512-token chunks starting at chunk g0,
                plus the afull slice load for the projection."""
                tok = 512 * ng
                a_in = dram.tile([NCORES * P, 64 * ng], bf16,
                                 name=f"ai{b}{g0}")
                a_out = dram.tile([NCORES * P, 64 * ng], bf16,
                                  name=f"ao{b}{g0}")
                av = a_in.rearrange("(j ee) t -> ee j t", j=NCORES)
                c0 = b * T + g0 * 512
                for h in range(HL):
                    nc.sync.dma_start(
                        av[64 * h:64 * h + 64],
                        anorm[h][:, c0:c0 + tok]
                        .rearrange("e (j t) -> e j t", j=NCORES),
                    )
                nc.gpsimd.collective_compute(
                    "AllToAll",
                    ALU.bypass,
                    replica_groups=[list(range(NCORES))],
                    ins=[a_in.opt()],
                    outs=[a_out.opt()],
                )
                part = g0 // 2
                if (b, part) not in afull:
                    afull[(b, part)] = apool.tile(
                        [P, NCORES, 128], bf16, tag="af",
                        name=f"af{b}{part}")
                t0 = (g0 % 2) * 64
                nc.gpsimd.dma_start(
                    afull[(b, part)][:, :, t0:t0 + 64 * ng],
                    a_out.rearrange("(i e) t -> e i t", i=NCORES),
                )

            def proj_units(b, part):
                """Filler units: output projection of one 128-token part."""
                units = []

                def mk(fc):
                    def u():
                        af = afull[(b, part)]
                        ps = mm.tile([P, 512], f32, tag="mm")
                        nc.tensor.matmul(
                            ps[:],
                            lhsT=ones_sb[:],
                            rhs=bp_sb[:, fc * 512:(fc + 1) * 512],
                            start=True,
                            stop=False,
                        )
                        for i in range(NCORES):
                            nc.tensor.matmul(
                                ps[:],
                                lhsT=af[:, i, :],
                                rhs=wp_sb[:, i, fc * 512:(fc + 1) * 512],
                                start=False,
                                stop=(i == NCORES - 1),
                            )
                        ysb = ypool.tile([P, 512], f32, tag="y")
                        nc.vector.tensor_copy(ysb[:], ps[:])
                        r0 = b * SL + part * P
                        nc.sync.dma_start(
                            y[r0:r0 + P, fc * 512:(fc + 1) * 512], ysb[:]
                        )
                    return u
                for fc in range(2):
                    units.append(mk(fc))
                return units

            def wp_load():
                nc.sync.dma_start(
                    wp_sb[:], wpT.rearrange("(ct p) f -> p ct f", p=P)
                )

            # ---- the schedule -------------------------------------------
            # a2a plan: coarse 1024-token collectives except the last two
            # chunks (512 tokens each) so the tail collective is small.
            for u in qkv_units(0, 0):
                u()
            plan = [
                (0, 0, qkv_units(0, 1), None),
                (0, 1, qkv_units(0, 2), (0, 0, 2)),
                (0, 2, qkv_units(0, 3) + [wp_load], None),
                (0, 3, qkv_units(1, 0), (0, 2, 2)),
                (1, 0, qkv_units(1, 1), None),
                (1, 1, qkv_units(1, 2), (1, 0, 2)),
                (1, 2, qkv_units(1, 3) + proj_units(0, 0), (1, 2, 1)),
                (1, 3, proj_units(0, 1) + proj_units(1, 0), (1, 3, 1)),
            ]
            for b, qc, fillers, xch in plan:
                o_t = att(b, qc, fillers)
                epi(b, qc, o_t)
                if xch is not None:
                    a2a(*xch)
            for u in proj_units(1, 1):
                u()
    nc.compile()
    return nc


def _prep_inputs(x, W_qkv, b_qkv, W_proj, b_proj):
    x = np.asarray(x, dtype=np.float32)
    W_qkv = np.asarray(W_qkv, dtype=np.float32)
    b_qkv = np.asarray(b_qkv, dtype=np.float32)
    W_proj = np.asarray(W_proj, dtype=np.float32)
    b_proj = np.asarray(b_proj, dtype=np.float32)

    import ml_dtypes
    bf = ml_dtypes.bfloat16
    xT = np.ascontiguousarray(x.reshape(TOK, C).T).astype(bf)
    wpT = np.ascontiguousarray(W_proj.T).astype(bf)
    tri = np.triu(np.ones((P, P), dtype=np.float32)).astype(bf)
    ident = np.eye(P, dtype=np.float32).astype(bf)
    ones = np.ones((P, P), dtype=np.float32).astype(bf)

    in_maps = []
    for p in range(NCORES):
        rows = np.r_[128 * p:128 * p + 128,
                     C + 128 * p:C + 128 * p + 128,
                     2 * C + 128 * p:2 * C + 128 * p + 128]
        wslice = W_qkv[rows]                      # [384, 1024]
        bslice = np.ascontiguousarray(b_qkv[rows])
        in_maps.append({
            "xT": xT,
            "wqkvT": np.ascontiguousarray(wslice.T).astype(bf),
            "bqkv": bslice,
            "wpT": wpT,
            "bp": b_proj.astype(bf),
            "tri": tri,
            "ident": ident,
            "ones": ones,
        })
    return in_maps


def kernel(x, W_qkv, b_qkv, W_proj, b_proj, _trace=False):
    from concourse import bass_utils

    if "nc" not in _CACHE:
        _CACHE["nc"] = _build_nc()
    nc = _CACHE["nc"]
    in_maps = _prep_inputs(x, W_qkv, b_qkv, W_proj, b_proj)
    res = bass_utils.run_bass_kernel_spmd(
        nc, in_maps, core_ids=list(range(NCORES)), trace=_trace,
    )
    _CACHE["last_result"] = res
    # Coarse parts (1024-token AllToAll): core p rows [b*256 + part*128 + t]
    # = batch b, token 1024*part + 128*p + t.  The final part (1,1) was
    # exchanged as two 512-token chunks: rows [256 + 128 + gg*64 + t] =
    # batch 1, token 512*(2+gg) + 64*p + t.
    yfull = np.empty((B, T, C), dtype=np.float32)
    for p, rmap in enumerate(res.results):
        yp = rmap["y"]
        for b in range(B):
            for part in range(2):
                if (b, part) != (1, 1):
                    g0 = part * 1024 + 128 * p
                    r0 = b * SL + part * P
                    yfull[b, g0:g0 + P] = yp[r0:r0 + P]
                else:
                    for gg in range(2):
                        r0 = b * SL + P + gg * 64
                        g0 = 512 * (2 + gg) + 64 * p
                        yfull[b, g0:g0 + 64] = yp[r0:r0 + 64]
    return yfull


# revision 19
# speedup vs baseline: 1.5165x; 1.5165x over previous
"""Causal self-attention (B=2, T=2048, C=1024, H=16) on 8 trn2 NeuronCores.

Sharding (Megatron-style over heads):
  - tensor-parallel over heads: core p owns heads {2p, 2p+1}.  Each core
    computes Q^T/K^T/V^T for its 2 heads from the full x, then causal
    attention (streaming softmax without max-subtraction; the denominator
    comes from a ones-column appended to V).
  - per 512-token q-chunk, an AllToAll redistributes that chunk's
    normalized attention outputs so core p holds all 1024 channels for
    tokens [512*g + 64*p, 512*g + 64*p + 64); the output projection runs
    per 128-token part (2 chunks) with the full W_proj.
  - projection output: core p writes a disjoint [512, 1024] block; rows
    b*256 + part*128 + gg*64 + t = batch b, token 512*(2*part+gg) + 64*p + t.

Scheduling: one software-pipelined instruction stream.  Attention per
q-chunk emits S(ki) -> [filler unit] -> exp(ki) -> mask -> PV(ki); the
filler units are the next batch's qkv chunks and earlier parts' output
projections, so the (in-order) PE never waits on the exp pipeline.  Exp
is sliced to the causal range [lo:512].  The softmax normalization is
fused into the PSUM->SBUF copy (o_t * recip broadcast -> anorm bf16).
DMA queues: bulk streams + recip bounce on sync, afull/y on scalar,
a2a staging + collectives on gpsimd.
"""

import numpy as np

B, T, C, H, D = 2, 2048, 1024, 16, 64
NCORES = 8
HL = H // NCORES        # heads per core = 2
TOK = B * T             # 4096 global tokens
TSL = TOK // NCORES     # 512 output tokens per core (256 per batch)
SL = 256                # per-batch token slice per core
P = 128
CT = C // P             # 8 contraction tiles
SCALE = D ** -0.5

_CACHE = {}


def _build_nc():
    import concourse.bass as bass
    import concourse.mybir as mybir
    from concourse import bacc
    from concourse.tile import TileContext

    f32 = mybir.dt.float32
    bf16 = mybir.dt.bfloat16
    AF = mybir.ActivationFunctionType
    ALU = mybir.AluOpType

    nc = bacc.Bacc(
        "TRN2", target_bir_lowering=False, debug=False, num_devices=NCORES
    )

    xT = nc.dram_tensor("xT", [C, TOK], bf16, kind="ExternalInput")
    wqkvT = nc.dram_tensor("wqkvT", [C, 3 * P], bf16, kind="ExternalInput")
    bqkv = nc.dram_tensor("bqkv", [3 * P], f32, kind="ExternalInput")
    wpT = nc.dram_tensor("wpT", [C, C], bf16, kind="ExternalInput")
    bp = nc.dram_tensor("bp", [C], bf16, kind="ExternalInput")
    tri = nc.dram_tensor("tri", [P, P], bf16, kind="ExternalInput")
    onesd = nc.dram_tensor("ones", [P, P], bf16, kind="ExternalInput")
    ident = nc.dram_tensor("ident", [P, P], bf16, kind="ExternalInput")
    y = nc.dram_tensor("y", [TSL, C], f32, kind="ExternalOutput")

    with TileContext(nc, num_cores=NCORES) as tc:
        from contextlib import ExitStack

        with ExitStack() as ctx:
            const = ctx.enter_context(tc.tile_pool(name="const", bufs=1))
            persist = ctx.enter_context(tc.tile_pool(name="persist", bufs=1))
            dram = ctx.enter_context(tc.tile_pool(name="dram", bufs=1, space="DRAM"))

            # ---- constants; small ones first so nothing queues behind bulk
            tri_sb = const.tile([P, P], bf16)
            id_sb = const.tile([P, P], bf16)
            bq_sb = const.tile([P, 3], f32)
            bp_sb = const.tile([1, C], bf16)
            ones_sb = const.tile([1, P], bf16)
            ones2_sb = const.tile([P, 2], bf16)
            w_sb = const.tile([P, CT, 3 * P], bf16)     # wqkvT tiles
            wp_sb = const.tile([P, CT, C], bf16)        # W_proj^T (loaded late)
            nc.gpsimd.dma_start(tri_sb[:], tri[:])
            nc.gpsimd.dma_start(id_sb[:], ident[:])
            nc.gpsimd.dma_start(bq_sb[:], bqkv.rearrange("(et p) -> p et", p=P))
            nc.gpsimd.dma_start(bp_sb[:], bp.rearrange("(o c) -> o c", o=1))
            nc.gpsimd.dma_start(ones_sb[:], onesd[0:1, :])
            nc.gpsimd.dma_start(ones2_sb[:], onesd[:, 0:2])
            nc.sync.dma_start(w_sb[:], wqkvT.rearrange("(ct p) e -> p ct e", p=P))

            # ---- persistent activations (per batch for fine-grained deps)
            qTb = [persist.tile([P, T], bf16, name=f"qT{b}") for b in range(B)]
            kTb = [persist.tile([P, T], bf16, name=f"kT{b}") for b in range(B)]
            vTb = [persist.tile([P, T], bf16, name=f"vT{b}") for b in range(B)]
            vaugb = [persist.tile([P, T // P, 2 * 65], bf16, name=f"vaug{b}")
                     for b in range(B)]
            anorm = [persist.tile([64, TOK], bf16, name=f"anorm{h}")
                     for h in range(HL)]
            rdram = dram.tile([B * HL, T], f32)          # reciprocals (bounce)

            pools = [
                tc.tile_pool(name="sps", bufs=2, space="PSUM"),
                tc.tile_pool(name="ops", bufs=2, space="PSUM"),
                tc.tile_pool(name="mm", bufs=2, space="PSUM"),
                tc.tile_pool(name="pT", bufs=2),
                tc.tile_pool(name="rp", bufs=2),
                tc.tile_pool(name="rb", bufs=2),
                tc.tile_pool(name="xslab", bufs=3),
                tc.tile_pool(name="afull", bufs=4),
                tc.tile_pool(name="ysb", bufs=2),
            ]
            (sps, ops, mm, ppool, rppool, rbpool, xpool, apool,
             ypool) = (ctx.enter_context(p) for p in pools)

            def qkv_units(b, c):
                """Filler units computing qkv^T for one 512-token chunk."""
                t0 = b * T + c * 512
                state = {}
                units = []

                def load():
                    xsl = xpool.tile([P, CT, 512], bf16, tag="x",
                                     name=f"x{b}{c}")
                    state["x"] = xsl
                    xv = xT[:, t0:t0 + 512].rearrange(
                        "(ct p) t -> p ct t", p=P)
                    nc.sync.dma_start(xsl[:, 0:4, :], xv[:, 0:4, :])
                    nc.sync.dma_start(xsl[:, 4:8, :], xv[:, 4:8, :])
                units.append(load)

                def mk_et(et, dstl):
                    def u():
                        ps = mm.tile([P, 512], f32, tag="mm")
                        for ct in range(CT):
                            nc.tensor.matmul(
                                ps[:],
                                lhsT=w_sb[:, ct, et * P:(et + 1) * P],
                                rhs=state["x"][:, ct, :],
                                start=(ct == 0),
                                stop=(ct == CT - 1),
                            )
                        nc.vector.tensor_scalar_add(
                            dstl[b][:, c * 512:(c + 1) * 512],
                            ps[:],
                            bq_sb[:, et:et + 1],
                        )
                    return u
                for et, dstl in enumerate((qTb, kTb, vTb)):
                    units.append(mk_et(et, dstl))

                def mk_tr(half):
                    def u():
                        for kt in range(c * 4 + half * 2, c * 4 + half * 2 + 2):
                            tp = mm.tile([P, P], bf16, tag="mm")
                            nc.tensor.transpose(
                                tp[:], vTb[b][:, kt * P:(kt + 1) * P], id_sb[:]
                            )
                            nc.vector.tensor_copy(
                                vaugb[b][:, kt, 0:2 * 65]
                                .rearrange("p (h e) -> p h e", h=2)[:, :, 0:64],
                                tp.rearrange("p (h e) -> p h e", h=2),
                            )
                            nc.vector.tensor_copy(
                                vaugb[b][:, kt, 64:2 * 65:65], ones2_sb[:]
                            )
                    return u
                for half in (0, 1):
                    units.append(mk_tr(half))
                return units

            def att(b, qc, fillers):
                """Causal attention for one 512-token q-chunk, weaving
                filler units between the score and PV matmul pairs."""
                q0 = qc * 512
                nk = 4 * qc + 4
                o_t = [ops.tile([65, 512], f32, tag="o", name=f"ot{h}")
                       for h in range(HL)]
                fi = 0
                for ki in range(nk):
                    sp = sps.tile([P, HL, 512], f32, tag="s")
                    for h in range(HL):
                        hp = slice(64 * h, 64 * h + 64)
                        nc.tensor.matmul(
                            sp[:, h, :],
                            lhsT=kTb[b][hp, ki * P:(ki + 1) * P],
                            rhs=qTb[b][hp, q0:q0 + 512],
                            start=True,
                            stop=True,
                        )
                    for _ in range(2 if ki == 0 else 1):
                        if fi < len(fillers):
                            fillers[fi]()
                            fi += 1
                    off = ki * P - q0
                    lo = max(0, off)
                    pt = ppool.tile([P, HL, 512], bf16, tag="p")
                    nc.scalar.activation(
                        pt[:, :, lo:512], sp[:, :, lo:512], AF.Exp, scale=SCALE,
                    )
                    for h in range(HL):
                        if off >= 0:
                            nc.vector.tensor_tensor(
                                pt[:, h, off:off + P],
                                pt[:, h, off:off + P],
                                tri_sb[:],
                                ALU.mult,
                            )
                        nc.tensor.matmul(
                            o_t[h][:, lo:512],
                            lhsT=vaugb[b][:, ki, h * 65:h * 65 + 65],
                            rhs=pt[:, h, lo:512],
                            start=(ki == 0),
                            stop=(ki == nk - 1),
                        )
                while fi < len(fillers):
                    fillers[fi]()
                    fi += 1
                return o_t

            def epi(b, qc, o_t):
                """Per-chunk softmax normalization (baseline structure):
                copy out the numerator and denominator rows first (releasing
                the PSUM bank), then reciprocal via a DRAM bounce for the
                partition reshape+broadcast, then multiply in place."""
                q0 = qc * 512
                c0 = b * T + q0
                for h in range(HL):
                    bh = b * HL + h
                    nc.vector.tensor_copy(
                        anorm[h][:, c0:c0 + 512], o_t[h][0:64, :],
                    )
                    dst = rppool.tile([65, 512], f32, tag="ds")
                    nc.vector.tensor_copy(dst[64:65, :], o_t[h][64:65, :])
                    dpk = rppool.tile([8, 64], f32, tag="dpk")
                    rpk = rppool.tile([8, 64], f32, tag="rpk")
                    rsc = rppool.tile([8, 64], f32, tag="rsc")
                    nc.sync.dma_start(dpk[:], dst[64:65, :])
                    nc.vector.reciprocal_approx_accurate(rpk[:], dpk[:], rsc[:])
                    nc.sync.dma_start(
                        rdram[bh:bh + 1, q0:q0 + 512]
                        .rearrange("o (rr f) -> (o rr) f", f=64),
                        rpk[:],
                    )
                    rb = rbpool.tile([64, 512], f32, tag="rb")
                    nc.sync.dma_start(
                        rb[:],
                        rdram[bh:bh + 1, q0:q0 + 512].to_broadcast((64, 512)),
                    )
                    nc.vector.tensor_tensor(
                        anorm[h][:, c0:c0 + 512],
                        anorm[h][:, c0:c0 + 512],
                        rb[:],
                        ALU.mult,
                    )

            afull = {}

            def a2a(b, g0, ng):
                """AllToAll for `ng` 512-token chunks starting at chunk g0,
                plus the afull slice load for the projection."""
                tok = 512 * ng
                a_in = dram.tile([NCORES * P, 64 * ng], bf16,
                                 name=f"ai{b}{g0}")
                a_out = dram.tile([NCORES * P, 64 * ng], bf16,
                                  name=f"ao{b}{g0}")
                av = a_in.rearrange("(j ee) t -> ee j t", j=NCORES)
                c0 = b * T + g0 * 512
                for h in range(HL):
                    nc.sync.dma_start(
                        av[64 * h:64 * h + 64],
                        anorm[h][:, c0:c0 + tok]
                        .rearrange("e (j t) -> e j t", j=NCORES),
                    )
                nc.gpsimd.collective_compute(
                    "AllToAll",
                    ALU.bypass,
                    replica_groups=[list(range(NCORES))],
                    ins=[a_in.opt()],
                    outs=[a_out.opt()],
                )
                part = g0 // 2
                if (b, part) not in afull:
                    afull[(b, part)] = apool.tile(
                        [P, NCORES, 128], bf16, tag="af",
                        name=f"af{b}{part}")
                t0 = (g0 % 2) * 64
                nc.gpsimd.dma_start(
                    afull[(b, part)][:, :, t0:t0 + 64 * ng],
                    a_out.rearrange("(i e) t -> e i t", i=NCORES),
                )

            def proj_units(b, part):
                """Filler units: output projection of one 128-token part."""
                units = []

                def mk(fc):
                    def u():
                        af = afull[(b, part)]
                        ps = mm.tile([P, 512], f32, tag="mm")
                        nc.tensor.matmul(
                            ps[:],
                            lhsT=ones_sb[:],
                            rhs=bp_sb[:, fc * 512:(fc + 1) * 512],
                            start=True,
                            stop=False,
                        )
                        for i in range(NCORES):
                            nc.tensor.matmul(
                                ps[:],
                                lhsT=af[:, i, :],
                                rhs=wp_sb[:, i, fc * 512:(fc + 1) * 512],
                                start=False,
                                stop=(i == NCORES - 1),
                            )
                        ysb = ypool.tile([P, 512], f32, tag="y")
                        nc.vector.tensor_copy(ysb[:], ps[:])
                        r0 = b * SL + part * P
                        nc.sync.dma_start(
                            y[r0:r0 + P, fc * 512:(fc + 1) * 512], ysb[:]
                        )
                    return u
                for fc in range(2):
                    units.append(mk(fc))
                return units

            def wp_load():
                nc.sync.dma_start(
                    wp_sb[:], wpT.rearrange("(ct p) f -> p ct f", p=P)
                )

            # ---- the schedule -------------------------------------------
            # a2a plan: coarse 1024-token collectives except the last two
            # chunks (512 tokens each) so the tail collective is small.
            for u in qkv_units(0, 0):
                u()
            plan = [
                (0, 0, qkv_units(0, 1), None),
                (0, 1, qkv_units(0, 2), (0, 0, 2)),
                (0, 2, qkv_units(0, 3) + [wp_load], None),
                (0, 3, qkv_units(1, 0), (0, 2, 2)),
                (1, 0, qkv_units(1, 1), None),
                (1, 1, qkv_units(1, 2), (1, 0, 2)),
                (1, 2, qkv_units(1, 3) + proj_units(0, 0), (1, 2, 1)),
                (1, 3, proj_units(0, 1) + proj_units(1, 0), (1, 3, 1)),
            ]
            for b, qc, fillers, xch in plan:
                o_t = att(b, qc, fillers)
                epi(b, qc, o_t)
                if xch is not None:
                    a2a(*xch)
            for u in proj_units(1, 1):
                u()
    nc.compile()
    return nc


def _prep_inputs(x, W_qkv, b_qkv, W_proj, b_proj):
    x = np.asarray(x, dtype=np.float32)
    W_qkv = np.asarray(W_qkv, dtype=np.float32)
    b_qkv = np.asarray(b_qkv, dtype=np.float32)
    W_proj = np.asarray(W_proj, dtype=np.float32)
    b_proj = np.asarray(b_proj, dtype=np.float32)

    import ml_dtypes
    bf = ml_dtypes.bfloat16
    xT = np.ascontiguousarray(x.reshape(TOK, C).T).astype(bf)
    wpT = np.ascontiguousarray(W_proj.T).astype(bf)
    tri = np.triu(np.ones((P, P), dtype=np.float32)).astype(bf)
    ident = np.eye(P, dtype=np.float32).astype(bf)
    ones = np.ones((P, P), dtype=np.float32).astype(bf)

    in_maps = []
    for p in range(NCORES):
        rows = np.r_[128 * p:128 * p + 128,
                     C + 128 * p:C + 128 * p + 128,
                     2 * C + 128 * p:2 * C + 128 * p + 128]
        wslice = W_qkv[rows]                      # [384, 1024]
        bslice = np.ascontiguousarray(b_qkv[rows])
        in_maps.append({
            "xT": xT,
            "wqkvT": np.ascontiguousarray(wslice.T).astype(bf),
            "bqkv": bslice,
            "wpT": wpT,
            "bp": b_proj.astype(bf),
            "tri": tri,
            "ident": ident,
            "ones": ones,
        })
    return in_maps


def kernel(x, W_qkv, b_qkv, W_proj, b_proj, _trace=False):
    from concourse import bass_utils

    if "nc" not in _CACHE:
        _CACHE["nc"] = _build_nc()
    nc = _CACHE["nc"]
    in_maps = _prep_inputs(x, W_qkv, b_qkv, W_proj, b_proj)
    res = bass_utils.run_bass_kernel_spmd(
        nc, in_maps, core_ids=list(range(NCORES)), trace=_trace,
    )
    _CACHE["last_result"] = res
    # Coarse parts (1024-token AllToAll): core p rows [b*256 + part*128 + t]
    # = batch b, token 1024*part + 128*p + t.  The final part (1,1) was
    # exchanged as two 512-token chunks: rows [256 + 128 + gg*64 + t] =
    # batch 1, token 512*(2+gg) + 64*p + t.
    yfull = np.empty((B, T, C), dtype=np.float32)
    for p, rmap in enumerate(res.results):
        yp = rmap["y"]
        for b in range(B):
            for part in range(2):
                if (b, part) != (1, 1):
                    g0 = part * 1024 + 128 * p
                    r0 = b * SL + part * P
                    yfull[b, g0:g0 + P] = yp[r0:r0 + P]
                else:
                    for gg in range(2):
                        r0 = b * SL + P + gg * 64
                        g0 = 512 * (2 + gg) + 64 * p
                        yfull[b, g0:g0 + 64] = yp[r0:r0 + 64]
    return yfull
